# revision 8
# baseline (speedup 1.0000x reference)
"""Trainium2 Bass kernel for nn_Attn (B=32, S=4096, H=1024, D=2*H=2048).

Reference computation:
    tmp      = einsum("bsd,hd->bsh", encoder_outputs, W) + b      # [B,S,H]
    energies = einsum("bh,bsh->bs", hidden, tmp)                  # [B,S]
    attn     = softmax(energies, axis=-1)[:, None, :]             # [B,1,S]

Key reassociation (exact in real arithmetic):
    energies[b,s] = enc[b,s,:] . v[b,:] + (hidden[b] . bias)
    with v[b,:] = hidden[b,:] @ W        # [B, D]
The bias term is constant over s, so it cancels inside softmax and is
dropped entirely.  This turns a 550-GFLOP dense matmul problem into a
memory-bound weighted-reduction stream over the 1 GiB encoder_outputs.

Sharding: data-parallel over batch for the enc stream (4 batches/core),
tensor-parallel over W's output dim for the tiny v precompute:
  1. each core loads only W[:, c*256:(c+1)*256] (1 MiB instead of the
     replicated 8 MiB) and computes v_part = hidden_all @ W_slice
     ([32, 256]) on TensorE,
  2. a 32 KiB AllToAll hands every core the v rows for its own 4
     batches across the full D=2048,
  3. v[b] is broadcast to 128 partitions with a rank-1 TensorE matmul
     (ones[1,128] outer v[b]) -- no DRAM roundtrip,
  4. enc tiles [128 s-partitions x SJ x 2048 d] stream on the sync
     HWDGE queue and reduce on DVE with fused scalar_tensor_tensor
     (out = in0 * in1, accum_out = row-sum) against the broadcast v,
  5. softmax runs per batch as soon as that batch's stream finishes
     (overlapped with later batches' streaming), entirely in the
     [128, 32] energy layout: per-partition max/exp/sum, then
     cross-partition max/sum via TensorE transpose-with-identity and
     ones-matmul broadcasts,
  6. each batch's attn [128, 32] tile DMAs straight to out[b] on the
     SWDGE queue (keeps the sync queue pure enc streaming).
"""

import numpy as np

import concourse.bacc as bacc
import concourse.tile as tile
from concourse import mybir
from concourse.bass_utils import run_bass_kernel_spmd

F32 = mybir.dt.float32

B, S, H, D = 32, 4096, 1024, 2048
NCORES = 8
BL = B // NCORES          # batches per core = 4
DSL = D // NCORES         # W output-dim slice per core = 256
KT = H // 128             # hidden k-tiles = 8
NJ = D // 512             # 512-wide N chunks in D for v broadcast = 4
SJ = 2                    # s-rows per partition per streamed DMA chunk
NQ = S // (128 * SJ)      # streamed DMA chunks per batch = 16
SCOLS = S // 128          # energy columns per partition = 32
STREAM_BUFS = 8


def build_bass():
    nc = bacc.Bacc()
    # hT[p, k*B + m] = hidden[m, k*128 + p]  (ALL batches; replicated)
    hT = nc.dram_tensor("hT", [128, KT * B], F32, kind="ExternalInput")
    # per-core W column slice W[:, c*DSL:(c+1)*DSL], contiguous
    wsl = nc.dram_tensor("wsl", [H, DSL], F32, kind="ExternalInput")
    enc = nc.dram_tensor("enc", [BL, S, D], F32, kind="ExternalInput")
    ident = nc.dram_tensor("ident", [128, 128], F32, kind="ExternalInput")
    ones = nc.dram_tensor("ones", [1, 128], F32, kind="ExternalInput")
    out = nc.dram_tensor("out", [BL, S], F32, kind="ExternalOutput")

    with tile.TileContext(nc) as tc:
        with (
            tc.tile_pool(name="persist", bufs=1) as persist,
            tc.tile_pool(name="stream", bufs=STREAM_BUFS) as stream,
            tc.tile_pool(name="psum_v", bufs=1, space="PSUM") as psum_v_pool,
            tc.tile_pool(name="psum_b", bufs=2, space="PSUM") as psum_b_pool,
            tc.tile_pool(name="psum_s", bufs=1, space="PSUM") as psum_s_pool,
            tc.tile_pool(name="dram", bufs=1, space="DRAM") as dram_pool,
        ):
            # ---- small loads first on the sync queue ----
            hT_sb = persist.tile([128, KT * B], F32, tag="hT")
            nc.sync.dma_start(out=hT_sb, in_=hT[:, :])
            w_sb = []
            for k in range(KT):
                wt = persist.tile([128, DSL], F32, tag=f"w{k}", name=f"w{k}")
                nc.sync.dma_start(out=wt, in_=wsl[k * 128:(k + 1) * 128, :])
                w_sb.append(wt)
            ident_sb = persist.tile([128, 128], F32, tag="ident")
            nc.sync.dma_start(out=ident_sb, in_=ident[:, :])
            ones_sb = persist.tile([1, 128], F32, tag="ones")
            nc.sync.dma_start(out=ones_sb, in_=ones[:, :])

            # ---- v_part = hidden_all @ W_slice -> [B, DSL] ----
            psv = psum_v_pool.tile([B, DSL], F32, tag="psv")
            for k in range(KT):
                nc.tensor.matmul(
                    psv,
                    hT_sb[:, k * B:(k + 1) * B],
                    w_sb[k],
                    start=(k == 0),
                    stop=(k == KT - 1),
                )
            v_part = persist.tile([B, DSL], F32, tag="vpart")
            nc.scalar.copy(out=v_part, in_=psv)

            # ---- AllToAll: v_part rows 4j..4j+3 go to core j; we get our
            # 4 batches' v across all 8 D-slices ----
            a2a_in = dram_pool.tile([B, DSL], F32, tag="a2ain")
            a2a_out = dram_pool.tile([B, DSL], F32, tag="a2aout")
            nc.gpsimd.dma_start(out=a2a_in, in_=v_part)
            nc.gpsimd.collective_compute(
                "AllToAll",
                mybir.AluOpType.bypass,
                replica_groups=[list(range(NCORES))],
                ins=[a2a_in[:, :]],
                outs=[a2a_out[:, :]],
            )
            # v_loc[b][0, r*DSL + i] = a2a_out[r*BL + b, i]; one [1, D] tile
            # per batch so each sits at partition base 0 (PE requirement).
            a2a_r = a2a_out[:, :].rearrange("(r m) i -> m r i", m=BL)
            v_loc = []
            for b in range(BL):
                vl = persist.tile([1, D], F32, tag=f"vloc{b}", name=f"vloc{b}")
                nc.gpsimd.dma_start(
                    out=vl[:, :].rearrange("m (r i) -> m r i", r=NCORES),
                    in_=a2a_r[b:b + 1],
                )
                v_loc.append(vl)

            # ---- broadcast v_loc[b] to 128 partitions via rank-1 matmul ----
            v_bc = []
            for b in range(BL):
                vb = persist.tile([128, D], F32, tag=f"vb{b}", name=f"vb{b}")
                for j in range(NJ):
                    pb = psum_b_pool.tile([128, 512], F32, tag="pbc")
                    nc.tensor.matmul(
                        pb,
                        ones_sb,
                        v_loc[b][:, j * 512:(j + 1) * 512],
                        start=True,
                        stop=True,
                    )
                    # alternate engines so consecutive chunk copies overlap
                    if j % 2 == 0:
                        nc.scalar.copy(out=vb[:, j * 512:(j + 1) * 512], in_=pb)
                    else:
                        nc.vector.tensor_copy(
                            out=vb[:, j * 512:(j + 1) * 512], in_=pb
                        )
                v_bc.append(vb)

            # ---- stream enc, fused multiply + row-reduce on DVE ----
            # s = p*SCOLS + q*SJ + j   (p = partition, column c = q*SJ + j)
            enc_r = enc[:, :, :].rearrange(
                "b (p q j) d -> b q p j d", p=128, q=NQ, j=SJ
            )
            e_tiles = [
                persist.tile([128, SCOLS], F32, tag=f"e{b}", name=f"e{b}")
                for b in range(BL)
            ]
            for b in range(BL):
                for q in range(NQ):
                    t = stream.tile([128, SJ, D], F32, tag="enc", name="enc_t")
                    nc.sync.dma_start(out=t, in_=enc_r[b, q])
                    for j in range(SJ):
                        # Fused multiply + add-reduce on DVE in one pass:
                        # out = (in0 * 1.0) * in1, accum_out = sum(out).
                        # out aliases in0 (the product is dead after the
                        # reduce).  NB: tensor_tensor_reduce wedges the device
                        # on this runtime path; scalar_tensor_tensor is the
                        # plain TENSOR_SCALAR_PTR ISA op and works.
                        nc.vector.scalar_tensor_tensor(
                            out=t[:, j, :],
                            in0=t[:, j, :],
                            scalar=1.0,
                            in1=v_bc[b],
                            op0=mybir.AluOpType.mult,
                            op1=mybir.AluOpType.mult,
                            accum_out=e_tiles[b][:, q * SJ + j:q * SJ + j + 1],
                        )

                # ---- per-batch softmax in the [128, SCOLS] layout,
                # overlapped with the next batch's streaming ----
                e = e_tiles[b]
                m_p = persist.tile([128, 1], F32, tag=f"mp{b}")
                nc.vector.tensor_reduce(
                    out=m_p, in_=e, axis=mybir.AxisListType.X,
                    op=mybir.AluOpType.max,
                )
                nm_p = persist.tile([128, 1], F32, tag=f"nmp{b}")
                nc.scalar.mul(out=nm_p, in_=m_p, mul=-1.0)
                s_p = persist.tile([128, 1], F32, tag=f"sp{b}")
                # e <- exp(e - m_p), s_p = row sums
                nc.scalar.activation(
                    out=e, in_=e,
                    func=mybir.ActivationFunctionType.Exp,
                    bias=nm_p, scale=1.0, accum_out=s_p,
                )
                # M = max_p m_p  (transpose via PE, reduce on DVE)
                mT = psum_s_pool.tile([1, 128], F32, tag="mT")
                nc.tensor.transpose(mT, m_p, ident_sb)
                mx = persist.tile([1, 1], F32, tag=f"mx{b}")
                nc.vector.tensor_reduce(
                    out=mx, in_=mT, axis=mybir.AxisListType.X,
                    op=mybir.AluOpType.max,
                )
                # -M broadcast to 128 partitions
                nmx = persist.tile([1, 1], F32, tag=f"nmx{b}")
                nc.scalar.mul(out=nmx, in_=mx, mul=-1.0)
                negMb = psum_s_pool.tile([128, 1], F32, tag="negMb")
                nc.tensor.matmul(
                    negMb, ones_sb, nmx[0:1, 0:1], start=True, stop=True
                )
                # w_p = exp(m_p - M)
                w_p = persist.tile([128, 1], F32, tag=f"wp{b}")
                nc.scalar.activation(
                    out=w_p, in_=negMb,
                    func=mybir.ActivationFunctionType.Exp,
                    bias=m_p, scale=1.0,
                )
                # Sw_p = s_p * w_p ; D = sum_p Sw_p
                sw_p = persist.tile([128, 1], F32, tag=f"swp{b}")
                nc.vector.scalar_tensor_tensor(
                    out=sw_p, in0=s_p, scalar=1.0, in1=w_p,
                    op0=mybir.AluOpType.mult, op1=mybir.AluOpType.mult,
                )
                swT = psum_s_pool.tile([1, 128], F32, tag="swT")
                nc.tensor.transpose(swT, sw_p, ident_sb)
                dsum = persist.tile([1, 1], F32, tag=f"ds{b}")
                nc.vector.tensor_reduce(
                    out=dsum, in_=swT, axis=mybir.AxisListType.X,
                    op=mybir.AluOpType.add,
                )
                rden = persist.tile([1, 1], F32, tag=f"rd{b}")
                nc.vector.reciprocal(out=rden, in_=dsum)
                rb = psum_s_pool.tile([128, 1], F32, tag="rb")
                nc.tensor.matmul(
                    rb, ones_sb, rden[0:1, 0:1], start=True, stop=True
                )
                # f_p = w_p * (1/D) ; attn = e * f_p
                f_p = persist.tile([128, 1], F32, tag=f"fp{b}")
                nc.vector.scalar_tensor_tensor(
                    out=f_p, in0=w_p, scalar=1.0, in1=rb,
                    op0=mybir.AluOpType.mult, op1=mybir.AluOpType.mult,
                )
                nc.vector.tensor_scalar_mul(e, e, f_p)
                # out[b, p*SCOLS + c] = e[p, c]; SWDGE queue keeps the sync
                # queue pure enc streaming.
                nc.gpsimd.dma_start(out=out[b:b + 1, :], in_=e[:, :])

    nc.compile()
    return nc


_NC_CACHE = None


def _get_nc():
    global _NC_CACHE
    if _NC_CACHE is None:
        _NC_CACHE = build_bass()
    return _NC_CACHE


def _make_in_maps(hidden, encoder_outputs, W):
    hidden = np.asarray(hidden, dtype=np.float32)
    encoder_outputs = np.asarray(encoder_outputs, dtype=np.float32)
    W = np.ascontiguousarray(np.asarray(W, dtype=np.float32))
    # hT[p, k*B + m] = hidden[m, k*128 + p]
    hT = np.ascontiguousarray(
        hidden.T.reshape(KT, 128, B).transpose(1, 0, 2).reshape(128, KT * B)
    )
    ident = np.eye(128, dtype=np.float32)
    ones = np.ones((1, 128), np.float32)
    in_maps = []
    for c in range(NCORES):
        in_maps.append({
            "hT": hT,
            "wsl": np.ascontiguousarray(W[:, c * DSL:(c + 1) * DSL]),
            "enc": np.ascontiguousarray(encoder_outputs[c * BL:(c + 1) * BL]),
            "ident": ident,
            "ones": ones,
        })
    return in_maps


def run_device(hidden, encoder_outputs, W, trace=False, **spmd_kwargs):
    nc = _get_nc()
    in_maps = _make_in_maps(hidden, encoder_outputs, W)
    res = run_bass_kernel_spmd(
        nc, in_maps, core_ids=list(range(NCORES)), trace=trace, **spmd_kwargs
    )
    outs = np.concatenate([r["out"] for r in res.results], axis=0)  # [B, S]
    return outs[:, None, :].astype(np.float32), res


def kernel(hidden, encoder_outputs, W, b):
    # `b` (the Linear bias) shifts every energy in a row equally
    # (hidden[b].bias, independent of s), so it cancels in the softmax.
    out, _ = run_device(hidden, encoder_outputs, W)
    return out
